# revision 5
# baseline (speedup 1.0000x reference)
"""QRNN forget-mult recurrence h_t = i_t*z_t + f_t*h_{t-1} on 8 NeuronCores.

Sharding: batch dim B=32 split 4-per-core (data parallel). Per core the
[T=4096, B=4, H=256] slice is viewed as C=1024 channels and pre-transposed
ON THE HOST to [C, T] float16 layout. On device, each 128-channel group is
one contiguous 1MB DMA; iz = i*z runs on DVE, and the recurrence runs as a
single tensor_tensor_scan over the full T=4096 free dim. The scan's internal
state is fp32 regardless of operand dtype, so fp16 I/O costs only the
input/output rounding (~8e-4 rel err against the 2e-2 gate; f in [0,1),
z ~ N(0,1) and |h| < ~30 all sit comfortably inside fp16 range). fp16 I/O
halves HBM traffic vs fp32: per-core 33.5MB -> ~93us DMA roofline at the
360 GB/s per-core HBM share, vs ~187us for fp32 I/O.

DMA queue assignment matters: h-out triggers wait on the scan, so they get
a dedicated queue (gpsimd SWDGE); f and z share the SP HWDGE queue and i
uses the Activation HWDGE queue so input prefetch never queues behind an
output trigger's semaphore wait.
"""

import numpy as np

T = 4096
B = 32
H = 256
NCORES = 8
BS = B // NCORES          # batches per core
C = BS * H                # channels per core
P = 128                   # partitions
NG = C // P               # channel groups per core

_CACHE = {}


def _build_nc(repeat=1, nchunks=1, ins_bufs=4, iz_bufs=3, ht_bufs=4,
              in_engines=("sync", "scalar", "sync"), out_engine="scalar",
              mul_engine="vector", scan_engine="vector", iz_dtype="io",
              io_dtype="fp16", out_delay=2):
    import concourse.tile as tile
    from concourse import bacc, mybir

    f32 = mybir.dt.float32
    f16 = (mybir.dt.float16 if io_dtype == "fp16" else mybir.dt.bfloat16)
    mult = mybir.AluOpType.mult
    add = mybir.AluOpType.add

    SC = T // nchunks         # timesteps per chunk

    nc = bacc.Bacc("TRN2", target_bir_lowering=False, debug=False)
    f_d = nc.dram_tensor("f", [C, T], f16, kind="ExternalInput")
    i_d = nc.dram_tensor("i", [C, T], f16, kind="ExternalInput")
    z_d = nc.dram_tensor("z", [C, T], f16, kind="ExternalInput")
    h0_d = nc.dram_tensor("h0", [P, NG], f32, kind="ExternalInput")
    h_d = nc.dram_tensor("h", [C, T], f16, kind="ExternalOutput")

    eng = lambda name: getattr(nc, name)

    with tile.TileContext(nc) as tc:
        with (
            tc.tile_pool(name="const", bufs=1) as constp,
            tc.tile_pool(name="ins", bufs=ins_bufs) as insp,
            tc.tile_pool(name="izp", bufs=iz_bufs) as izp,
            tc.tile_pool(name="hts", bufs=ht_bufs) as htp,
        ):
            h0t = constp.tile([P, NG], f32)
            nc.sync.dma_start(h0t[:], h0_d[:, :])

            prev_ht = [None] * NG
            # h-out triggers go on an HWDGE queue (faster than gpsimd
            # SWDGE), issued out_delay group-iterations late so their
            # wait-on-scan semaphore is already satisfied when the queue
            # head reaches them — input prefetch behind them never stalls.
            pending = []
            for r in range(repeat):
                for ch in range(nchunks):
                    ts = slice(ch * SC, (ch + 1) * SC)
                    for g in range(NG):
                        gs = slice(g * P, (g + 1) * P)
                        ft = insp.tile([P, SC], f16, tag="f")
                        eng(in_engines[0]).dma_start(ft[:], f_d[gs, ts])
                        it = insp.tile([P, SC], f16, tag="i")
                        eng(in_engines[1]).dma_start(it[:], i_d[gs, ts])
                        zt = insp.tile([P, SC], f16, tag="z")
                        eng(in_engines[2]).dma_start(zt[:], z_d[gs, ts])
                        izt = izp.tile([P, SC],
                                       f32 if iz_dtype == "f32" else f16,
                                       tag="iz")
                        eng(mul_engine).tensor_mul(izt[:], it[:], zt[:])
                        ht = htp.tile([P, SC], f16, tag="h")
                        init = (h0t[:, g:g + 1] if ch == 0
                                else prev_ht[g][:, SC - 1:SC])
                        eng(scan_engine).tensor_tensor_scan(
                            ht[:], ft[:], izt[:], init, op0=mult, op1=add)
                        prev_ht[g] = ht
                        pending.append((gs, ts, ht))
                        if len(pending) > out_delay:
                            pgs, pts, pht = pending.pop(0)
                            eng(out_engine).dma_start(h_d[pgs, pts], pht[:])
            for pgs, pts, pht in pending:
                eng(out_engine).dma_start(h_d[pgs, pts], pht[:])

    nc.compile()
    return nc


def _get_nc():
    if "nc" not in _CACHE:
        _CACHE["nc"] = _build_nc()
    return _CACHE["nc"]


def make_in_maps(f, z, i, hidden_init, np_dtype=np.float16):
    # One global [T, B*H] -> [B*H, T] fp16 transpose per tensor; per-core
    # slices are then zero-copy contiguous views (channel c = b*H + h).
    def to_ct(x):
        x = np.asarray(x, dtype=np.float32).reshape(T, B * H).astype(np_dtype)
        return np.ascontiguousarray(x.T)

    fT, iT, zT = to_ct(f), to_ct(i), to_ct(z)
    h0 = np.asarray(hidden_init, dtype=np.float32).reshape(B * H)
    in_maps = []
    for c in range(NCORES):
        c0 = c * C
        in_maps.append({
            "f": fT[c0:c0 + C],
            "i": iT[c0:c0 + C],
            "z": zT[c0:c0 + C],
            # [C] -> [P, NG]: tile h0t[p, g] holds channel g*P + p
            "h0": np.ascontiguousarray(h0[c0:c0 + C].reshape(NG, P).T),
        })
    return in_maps


def kernel(f, z, i, hidden_init):
    import time

    from concourse.bass_utils import run_bass_kernel_spmd

    in_maps = make_in_maps(f, z, i, hidden_init)
    last_err = None
    for attempt in range(3):
        try:
            res = run_bass_kernel_spmd(
                _get_nc(), in_maps, list(range(NCORES))
            ).results
            break
        except Exception as e:  # transient device-unrecoverable states
            last_err = e
            time.sleep(2.0 * (attempt + 1))
    else:
        raise last_err
    out = np.empty((T, B, H), np.float32)
    for c in range(NCORES):
        # [C, T] fp16 -> [T, BS, H] fp32
        hct = np.asarray(res[c]["h"]).reshape(BS, H, T)
        out[:, c * BS:(c + 1) * BS, :] = hct.transpose(2, 0, 1).astype(np.float32)
    return out
